# revision 1
# baseline (speedup 1.0000x reference)
"""Trainium2 Bass kernel for nn_Min_interval (subset min-interval selection).

Problem: for each batch row, for every subset S of the 16 input columns with
|S| <= 3, output the (xl, xu) interval of the column in S minimizing the
score s = 0.5*xl + 0.5*xu (ties broken by b = 0.2*xl + 0.8*xu, then by DP
fold order).  Output columns are sorted by subset bitmask -> [B, 696] x 2.

Device algorithm (per core, pure data-parallel over rows):
  *  s~ = xl + xu compares identically to 0.5*xl+0.5*xu (exact halving), so
     the kernel compares s~ and the winner of any subset is argmin s~.
  *  M2 = the 136 subsets with |S|<=2 in bitmask order have a prefix
     structure: block j = [single j, pair(0,j), .., pair(j-1,j)] and every
     pair block's left operands are the contiguous singles prefix.
  *  The full output in bitmask order is block t = [single t] ++
     [select(M2[k], single t) for k < c2(t)], where c2(t) = t(t+1)/2 is the
     number of |S|<=2 subsets with bitmask < 2^t.  So every output block is
     one uniform select of a *contiguous prefix* of the M2 staging array
     against one broadcast column.
  *  A sentinel column with s~ = +inf is prepended to both the input staging
     and M2 staging arrays; select(sentinel, single t) = single t, which
     makes the leading "single" element of every block fall out of the same
     vectorized op (no scattered scalar copies anywhere).
  *  Each select is: is_gt mask (Vector engine), plain copy of the on-false
     prefix (Scalar engine, merged l/u/s~ planes via 4-dim APs), and
     copy_predicated of the broadcast on-true column (Vector engine).  The
     s~ plane of the select result doubles as min(left,right), feeding the
     next stage.
  *  Outputs stream to HBM in two column waves per chunk, split across the
     SP-HWDGE and SWDGE DMA queues so stores overlap compute and the next
     chunk's input loads.
  *  Exact s~ ties between distinct columns (~1e-7 of rows) are detected on
     host and those rows are recomputed with exact reference semantics in
     numpy, so the device kernel needs strict > only.

Sharding: batch 65536 rows -> 8 cores x 8192 rows (data parallel, no comms).
Measured: ~204 us/core CoreSim estimate, ~200 us/core on HW (per-core HBM
write roofline for the 45.6 MB output slice is ~127 us; the Vector engine is
the bottleneck at ~100% busy — it owns all compares and predicated copies).
"""

import os
import sys
import numpy as np

for _p in ("/opt/trn_rl_repo",):
    if _p not in sys.path and os.path.isdir(_p):
        sys.path.insert(0, _p)

# ----------------------------------------------------------------------------
# Problem constants (hardcoded per contest rules)
# ----------------------------------------------------------------------------
N = 16                 # input feature columns
ADD = 3                # max subset order
ALPHA = 0.5
BETA = 0.8
BATCH = 65536
N_CORES = 8
ROWS_PER_CORE = BATCH // N_CORES        # 8192
P = 128                                 # SBUF partitions
OUT_COLS = 696                          # subsets with 1<=|S|<=3 of 16
NB_DEFAULT = 16                         # rowblocks per chunk

# triangular prefix counts: c2[t] = #subsets |S|<=2 with bitmask < 2^t
C2 = [t * (t + 1) // 2 for t in range(N + 1)]
# output block offsets: block t (top bit t) starts at BOFF[t], width 1+c2[t]
BOFF = [0] * (N + 1)
for _t in range(N):
    BOFF[_t + 1] = BOFF[_t] + 1 + C2[_t]
assert BOFF[N] == OUT_COLS

M2_COLS = C2[N]  # 136 = number of |S|<=2 subsets (120 pairs + 16 singles)

# SBUF staging layouts (all rowblock-major along the free dim)
C_INQ = N + 1            # 17: sentinel + 16 input cols, per plane
C_IN = 3 * C_INQ         # 51: l, u, s~ planes
C_P2Q = 1 + M2_COLS      # 137: sentinel + M2
C_P2 = 3 * C_P2Q         # 411
C_OUT = 2 * OUT_COLS     # 1392


# ----------------------------------------------------------------------------
# Bass program builder
# ----------------------------------------------------------------------------
def build_program(rows=ROWS_PER_CORE, nb=NB_DEFAULT, reps=1):
    """Build the per-core Bass program. rows must be divisible by 128*nb.

    reps repeats the whole computation in-program (benchmarking only).
    """
    from contextlib import ExitStack
    from concourse import bacc, mybir, tile

    f32 = mybir.dt.float32
    u32 = mybir.dt.uint32
    gt = mybir.AluOpType.is_gt

    chunks = rows // (P * nb)
    assert chunks * P * nb == rows

    nc = bacc.Bacc()
    xl_d = nc.declare_dram_parameter("xl", [rows, N], f32, isOutput=False)
    xu_d = nc.declare_dram_parameter("xu", [rows, N], f32, isOutput=False)
    ol_d = nc.declare_dram_parameter("out_l", [rows, OUT_COLS], f32, isOutput=True)
    ou_d = nc.declare_dram_parameter("out_u", [rows, OUT_COLS], f32, isOutput=True)

    # DRAM views: (chunk, partition, rowblock, col)
    xl_r = xl_d[:].rearrange("(c nb p) t -> c p nb t", nb=nb, p=P)
    xu_r = xu_d[:].rearrange("(c nb p) t -> c p nb t", nb=nb, p=P)
    ol_r = ol_d[:].rearrange("(c nb p) o -> c p nb o", nb=nb, p=P)
    ou_r = ou_d[:].rearrange("(c nb p) o -> c p nb o", nb=nb, p=P)

    out_bufs = 2 if nb <= 8 else 1
    with ExitStack() as ctx:
        tc = ctx.enter_context(tile.TileContext(nc))
        inp = ctx.enter_context(tc.tile_pool(name="inp", bufs=2))
        p2p = ctx.enter_context(tc.tile_pool(name="p2p", bufs=2))
        outp = ctx.enter_context(tc.tile_pool(name="outp", bufs=out_bufs))
        outpB = ctx.enter_context(tc.tile_pool(name="outpB", bufs=1))
        mp = ctx.enter_context(tc.tile_pool(name="mp", bufs=3))

        for _rep in range(reps):
          for ch in range(chunks):
            inb = inp.tile([P, nb * C_IN], f32, tag="inb")
            # [p, nb, c] and [p, v, nb, q] views of the input staging tile
            in3 = inb[:].rearrange("p (nb c) -> p nb c", c=C_IN)
            in4 = inb[:].rearrange("p (nb v q) -> p v nb q", v=3, q=C_INQ)

            # load xl -> l plane cols 1..16, xu -> u plane cols 1..16
            nc.sync.dma_start(out=in3[:, :, 1:1 + N], in_=xl_r[ch])
            nc.sync.dma_start(out=in3[:, :, C_INQ + 1:C_INQ + 1 + N], in_=xu_r[ch])

            # s~ plane: sentinel = +inf, cols 1..16 = l + u
            soff = 2 * C_INQ
            nc.gpsimd.memset(in3[:, :, soff:soff + 1], float("inf"))
            nc.vector.tensor_add(
                in3[:, :, soff + 1:soff + 1 + N],
                in3[:, :, 1:1 + N],
                in3[:, :, C_INQ + 1:C_INQ + 1 + N],
            )

            p2 = p2p.tile([P, nb * C_P2], f32, tag="p2")
            p23 = p2[:].rearrange("p (nb c) -> p nb c", c=C_P2)
            p24 = p2[:].rearrange("p (nb v q) -> p v nb q", v=3, q=C_P2Q)
            s2off = 2 * C_P2Q
            nc.gpsimd.memset(p23[:, :, s2off:s2off + 1], float("inf"))

            # ---------------- pairs stage: fill M2 staging ----------------
            # group j writes M2 block j = [single j, pair(0,j)..pair(j-1,j)]
            # at q = 1+c2(j) .. 1+c2(j)+j   (q=0 is the sentinel)
            for j in range(N):
                W = j + 1
                q0 = 1 + C2[j]
                # left operand: sentinel + singles 0..j-1  (s~ cols 0..j)
                ls = in3[:, :, soff:soff + W]
                # broadcast right operand: single j
                rs = in3[:, :, soff + 1 + j:soff + 2 + j].to_broadcast((P, nb, W))

                pm = mp.tile([P, nb * N], u32, tag="pm")
                pm3 = pm[:].rearrange("p (nb w) -> p nb w", w=N)[:, :, :W]
                nc.vector.tensor_tensor(pm3, ls, rs, gt)

                # l,u,s~ of winners in one shot: copy left prefix, overwrite
                # with right where mask (v=3 planes via 4-dim APs).  The s~
                # plane's select result equals min(left,right) since the mask
                # is exactly (left > right).
                dst = p24[:, 0:3, :, q0:q0 + W]
                nc.scalar.copy(dst, in4[:, 0:3, :, 0:W])
                data = in4[:, 0:3, :, 1 + j:2 + j].to_broadcast((P, 3, nb, W))
                maskb = pm3.unsqueeze(1).to_broadcast((P, 3, nb, W))
                nc.vector.copy_predicated(dst, maskb, data)

            # ---------------- final stage: emit output blocks ----------------
            # two wave tensors so wave-A DMAs overlap wave-B compute and the
            # next chunk can start on wave A while wave B drains
            T_SPLIT = 14
            wA = BOFF[T_SPLIT]
            wB = OUT_COLS - wA
            osbA = outp.tile([P, nb * 2 * wA], f32, tag="osbA")
            osbB = outpB.tile([P, nb * 2 * wB], f32, tag="osbB")
            o4A = osbA[:].rearrange("p (nb v c) -> p v nb c", v=2, c=wA)
            o4B = osbB[:].rearrange("p (nb v c) -> p v nb c", v=2, c=wB)

            for t in range(N):
                W = C2[t] + 1
                b0 = BOFF[t]
                ls = p23[:, :, s2off:s2off + W]
                rs = in3[:, :, soff + 1 + t:soff + 2 + t].to_broadcast((P, nb, W))

                fm = mp.tile([P, nb * (C2[N - 1] + 1)], u32, tag="fm")
                fm3 = fm[:].rearrange("p (nb w) -> p nb w", w=C2[N - 1] + 1)[:, :, :W]
                nc.vector.tensor_tensor(fm3, ls, rs, gt)

                if t < T_SPLIT:
                    dst = o4A[:, :, :, b0:b0 + W]
                else:
                    dst = o4B[:, :, :, b0 - wA:b0 - wA + W]
                nc.scalar.copy(dst, p24[:, 0:2, :, 0:W])
                data = in4[:, 0:2, :, 1 + t:2 + t].to_broadcast((P, 2, nb, W))
                maskb = fm3.unsqueeze(1).to_broadcast((P, 2, nb, W))
                nc.vector.copy_predicated(dst, maskb, data)

                if t == T_SPLIT - 1:
                    # wave A out-DMAs overlap wave-B compute
                    nc.sync.dma_start(out=ol_r[ch][:, :, :wA], in_=o4A[:, 0])
                    nc.gpsimd.dma_start(out=ou_r[ch][:, :, :wA], in_=o4A[:, 1])

            nc.sync.dma_start(out=ol_r[ch][:, :, wA:], in_=o4B[:, 0])
            nc.gpsimd.dma_start(out=ou_r[ch][:, :, wA:], in_=o4B[:, 1])

    nc.finalize()
    return nc


# ----------------------------------------------------------------------------
# Exact reference semantics in numpy (for rare s~ tie rows)
# ----------------------------------------------------------------------------
def _build_plan():
    from itertools import combinations

    items = list(range(N))
    index_dict = {(i,): i for i in items}
    count = N
    plan = []
    for length in range(2, min(ADD, N) + 1):
        combos = list(combinations(items, length))
        left = np.array([index_dict[c[1:]] for c in combos], dtype=np.int32)
        right = np.array([index_dict[c[:-1]] for c in combos], dtype=np.int32)
        for c in combos:
            index_dict[c] = count
            count += 1
        plan.append((left, right))

    def bitmask(c):
        m = 0
        for i in c:
            m |= 1 << i
        return m

    order = np.array(
        [index_dict[c] for c in sorted(index_dict, key=bitmask)], dtype=np.int32
    )
    return plan, order


_PLAN_CACHE = None


def _reference_numpy(xl, xu):
    """Bit-exact fp32 reproduction of the jax reference for given rows."""
    global _PLAN_CACHE
    if _PLAN_CACHE is None:
        _PLAN_CACHE = _build_plan()
    plan, order = _PLAN_CACHE
    a0 = np.float32(1.0 - ALPHA)
    a1 = np.float32(ALPHA)
    b0 = np.float32(1.0 - BETA)
    b1 = np.float32(BETA)
    mat_l, mat_u = xl.astype(np.float32), xu.astype(np.float32)
    for left_idx, right_idx in plan:
        ll, lu = mat_l[:, left_idx], mat_u[:, left_idx]
        rl, ru = mat_l[:, right_idx], mat_u[:, right_idx]
        cur = a0 * ll + a1 * lu
        nxt = a0 * rl + a1 * ru
        bcur = b0 * ll + b1 * lu
        bnxt = b0 * rl + b1 * ru
        choose_right = np.where(cur == nxt, bcur > bnxt, cur > nxt)
        res_l = np.where(choose_right, rl, ll)
        res_u = np.where(choose_right, ru, lu)
        mat_l = np.concatenate([mat_l, res_l], axis=1)
        mat_u = np.concatenate([mat_u, res_u], axis=1)
    return mat_l[:, order], mat_u[:, order]


# ----------------------------------------------------------------------------
# Host entry point
# ----------------------------------------------------------------------------
_PROGRAM_CACHE = {}


def _get_program(rows, nb):
    key = (rows, nb)
    if key not in _PROGRAM_CACHE:
        _PROGRAM_CACHE[key] = build_program(rows, nb)
    return _PROGRAM_CACHE[key]


def kernel(xl, xu):
    from concourse.bass_utils import run_bass_kernel_spmd

    xl = np.ascontiguousarray(np.asarray(xl), dtype=np.float32)
    xu = np.ascontiguousarray(np.asarray(xu), dtype=np.float32)
    assert xl.shape == (BATCH, N) and xu.shape == (BATCH, N)

    nc = _get_program(ROWS_PER_CORE, NB_DEFAULT)

    in_maps = []
    for c in range(N_CORES):
        sl = slice(c * ROWS_PER_CORE, (c + 1) * ROWS_PER_CORE)
        in_maps.append({"xl": xl[sl], "xu": xu[sl]})

    res = run_bass_kernel_spmd(nc, in_maps, list(range(N_CORES))).results

    out_l = np.concatenate([r["out_l"] for r in res], axis=0)
    out_u = np.concatenate([r["out_u"] for r in res], axis=0)

    # Patch rows where two distinct columns have exactly equal s~ keys: the
    # device kernel uses strict-> only; the reference tie-breaks via beta
    # score and DP fold order.  (~1e-7 of rows; exact recompute on host.)
    s = xl + xu
    ss = np.sort(s, axis=1)
    bad = (np.diff(ss, axis=1) == 0).any(axis=1)
    rows = np.nonzero(bad)[0]
    if rows.size:
        pl, pu = _reference_numpy(xl[rows], xu[rows])
        out_l[rows] = pl
        out_u[rows] = pu

    return out_l, out_u

